# revision 1
# baseline (speedup 1.0000x reference)
"""Trainium2 Bass kernel for nn_Attention_65798898975102.

AdaNorm(RMS) -> QKV -> per-head RMS(q,k) -> RoPE -> softmax attention -> out proj.
B=4, L=2048, H=1024, D=64, NH=16 heads. 8 NeuronCores, each handling one
(batch, head-group-of-8) shard; host sums the two head-group partial outputs
per batch (W_out row-split all-reduce done on host).

Numerics: fp32r matmuls (measured ~1.5e-4 rel err, bf16 speed), exact-softmax
without max subtraction (|logit| <= sqrt(D) bound after q/k RMS norm), Z via
ones-column augmented V.
"""
import sys

sys.path.insert(0, "/opt/trn_rl_repo")

import numpy as np
from contextlib import ExitStack

B, L, H, C, D = 4, 2048, 1024, 1024, 64
NH = 16
EPS = 1e-6
P = 128
NCORES = 8
HG = 2            # head groups per batch
HPG = NH // HG    # 8 heads per group
GD = HPG * D      # 512 feature dims per group
KC = H // P       # 8 contraction chunks over H
LQ = 4            # Lq chunks of 512
LK = L // P       # 16 Lk chunks of 128
F32 = None        # set after imports
VA_W = HPG * (D + 1) + 8   # 528 (65 per head, padded to 528)

_compiled = [None]
DEBUG = False


def _build():
    import concourse.mybir as mybir
    import concourse.bacc as bacc
    import concourse.tile as tile

    f32 = mybir.dt.float32
    f32r = mybir.dt.float32r
    AF = mybir.ActivationFunctionType
    OP = mybir.AluOpType

    nc = bacc.Bacc("TRN2", target_bir_lowering=False, debug=False,
                   num_devices=NCORES)

    # ---- DRAM I/O ----
    xT = nc.dram_tensor("xT", [H, L], f32, kind="ExternalInput").ap()
    wqk = nc.dram_tensor("wqk", [H, 2 * GD], f32, kind="ExternalInput").ap()
    wv = nc.dram_tensor("wv", [H, GD], f32, kind="ExternalInput").ap()
    wout = nc.dram_tensor("wout", [GD, H], f32, kind="ExternalInput").ap()
    bqk = nc.dram_tensor("bqk", [P, 8], f32, kind="ExternalInput").ap()
    vb = nc.dram_tensor("vb", [P, GD], f32, kind="ExternalInput").ap()
    ropes = nc.dram_tensor("ropes", [4, P, L], f32, kind="ExternalInput").ap()
    prot = nc.dram_tensor("prot", [P, P], f32, kind="ExternalInput").ap()
    bo2 = nc.dram_tensor("bo2", [P, 2], f32, kind="ExternalInput").ap()
    ones8 = nc.dram_tensor("ones8", [P, 8], f32, kind="ExternalInput").ap()
    out = nc.dram_tensor("out", [L, H], f32, kind="ExternalOutput").ap()
    dbg = {}
    if DEBUG:
        for nm, shp in [("dxn", [H, L]), ("dqk", [H, L]), ("dqkn", [H, L]),
                        ("dqkr", [H, L]), ("dva", [P * LK, VA_W]),
                        ("don", [GD, L])]:
            dbg[nm] = nc.dram_tensor(nm, shp, f32,
                                     kind="ExternalOutput").ap()

    with tile.TileContext(nc) as tc, ExitStack() as octx:
        consts = octx.enter_context(
            tc.tile_pool(name="consts", bufs=1, side="left"))
        prot_t = consts.tile([P, P], f32r)
        nc.sync.dma_start(prot_t[:], prot[:].bitcast(f32r))
        bo2_t = consts.tile([P, 2], f32r)
        nc.sync.dma_start(bo2_t[:], bo2[:].bitcast(f32r))
        bqk_t = consts.tile([P, 8], f32)
        nc.sync.dma_start(bqk_t[:], bqk[:])
        vb_t = consts.tile([P, GD], f32)
        nc.sync.dma_start(vb_t[:], vb[:])
        eps_t = consts.tile([P, 1], f32)
        nc.vector.memset(eps_t[:], EPS)
        ones8_t = consts.tile([P, 8], f32r)
        nc.sync.dma_start(ones8_t[:], ones8[:].bitcast(f32r))

        qk_pool = octx.enter_context(
            tc.tile_pool(name="qk", bufs=1, side="left"))
        qkt = [qk_pool.tile([P, L], f32r, tag=f"qkt{t}", name=f"qkt{t}")
               for t in range(8)]

        with ExitStack() as xctx:
            xh_pool = xctx.enter_context(
                tc.tile_pool(name="xh", bufs=1, side="right"))
            xh = []
            for j in range(KC):
                t = xh_pool.tile([P, L], f32r, tag=f"xh{j}", name=f"xh{j}")
                nc.sync.dma_start(t[:], xT[j * P:(j + 1) * P, :].bitcast(f32r))
                xh.append(t)

            # ---------- phase 1: x row rms norm + adanorm ----------
            with tc.tile_pool(name="ph1", bufs=2, side="right") as ph1, \
                 tc.tile_pool(name="ph1r", bufs=1, side="right") as ph1r, \
                 tc.tile_pool(name="ph1ps", bufs=1, space="PSUM",
                              side="right") as ph1ps:
                stat_ps = [ph1ps.tile([1, 512], f32, tag=f"sps{c}",
                                      name=f"sps{c}") for c in range(LQ)]
                for j in range(KC):
                    for c in range(LQ):
                        xsq = ph1.tile([P, 512], f32r, tag="xsq")
                        seg = xh[j][:, c * 512:(c + 1) * 512]
                        nc.vector.tensor_mul(xsq[:], seg, seg)
                        nc.tensor.matmul(stat_ps[c][:], lhsT=ones8_t[:, 0:1],
                                         rhs=xsq[:], start=(j == 0),
                                         stop=(j == KC - 1))
                srow = ph1r.tile([1, L], f32, tag="srow")
                rrow = ph1r.tile([1, L], f32, tag="rrow")
                rbc = ph1r.tile([P, L], f32, tag="rbc")
                for c in range(LQ):
                    nc.scalar.activation(srow[:, c * 512:(c + 1) * 512],
                                         stat_ps[c][:], AF.Sqrt,
                                         bias=eps_t[0:1, :], scale=1.0 / H)
                nc.vector.reciprocal_approx_fast(out=rrow[:], in_=srow[:])
                nc.gpsimd.partition_broadcast(rbc[:], rrow[:])
                # xn^T = x^T * r  (adanorm scale/shift folded into weights)
                for j in range(KC):
                    nc.vector.tensor_mul(xh[j][:], xh[j][:], rbc[:])

            if DEBUG:
                for j in range(KC):
                    nc.sync.dma_start(
                        dbg["dxn"][j * P:(j + 1) * P, :],
                        xh[j][:].bitcast(f32))

            # ---------- phase 2a: q,k projection (feature-major) ----------
            with tc.tile_pool(name="wqkp", bufs=2, side="right") as wqkp, \
                 tc.tile_pool(name="qkps", bufs=3, space="PSUM",
                              side="right") as qkps:
                for cc in range(8):
                    wt = []
                    for kk in range(KC):
                        w = wqkp.tile([P, P], f32r, tag=f"w{kk}", name=f"w{kk}")
                        nc.sync.dma_start(
                            w[:], wqk[kk * P:(kk + 1) * P,
                                      cc * P:(cc + 1) * P].bitcast(f32r))
                        wt.append(w)
                    for lq in range(LQ):
                        ps = qkps.tile([P, 512], f32, tag="qkps")
                        for kk in range(KC):
                            nc.tensor.matmul(
                                ps[:], lhsT=wt[kk][:],
                                rhs=xh[kk][:, lq * 512:(lq + 1) * 512],
                                start=(kk == 0), stop=(kk == KC - 1))
                        nc.vector.tensor_scalar_add(
                            out=qkt[cc][:, lq * 512:(lq + 1) * 512],
                            in0=ps[:], scalar1=bqk_t[:, cc:cc + 1])

            if DEBUG:
                for t in range(8):
                    nc.sync.dma_start(
                        dbg["dqk"][t * P:(t + 1) * P, :],
                        qkt[t][:].bitcast(f32))

            # ---------- phase 2b: q,k per-head rms norm ----------
            with tc.tile_pool(name="nsc", bufs=3, side="right") as nsc, \
                 tc.tile_pool(name="nrow", bufs=1, side="right") as nrow, \
                 tc.tile_pool(name="nps", bufs=4, space="PSUM",
                              side="right") as nps:
                for t in range(8):
                    sr0 = nrow.tile([1, L], f32, tag="sr0")
                    sr1 = nrow.tile([1, L], f32, tag="sr1")
                    for c in range(LQ):
                        cs = slice(c * 512, (c + 1) * 512)
                        seg = qkt[t][:, cs]
                        qsq = nsc.tile([P, 512], f32r, tag="qsq")
                        nc.vector.tensor_mul(qsq[:], seg, seg)
                        sp0 = nps.tile([1, 512], f32, tag="sp0")
                        sp1 = nps.tile([1, 512], f32, tag="sp1")
                        nc.tensor.matmul(sp0[:], lhsT=bo2_t[:, 0:1],
                                         rhs=qsq[:], start=True, stop=True)
                        nc.tensor.matmul(sp1[:], lhsT=bo2_t[:, 1:2],
                                         rhs=qsq[:], start=True, stop=True)
                        nc.scalar.activation(sr0[:, cs], sp0[:], AF.Sqrt,
                                             bias=eps_t[0:1, :], scale=1.0 / D)
                        nc.scalar.activation(sr1[:, cs], sp1[:], AF.Sqrt,
                                             bias=eps_t[0:1, :], scale=1.0 / D)
                    r0 = nrow.tile([1, L], f32, tag="r0")
                    r1 = nrow.tile([1, L], f32, tag="r1")
                    nc.vector.reciprocal_approx_fast(out=r0[:], in_=sr0[:])
                    nc.vector.reciprocal_approx_fast(out=r1[:], in_=sr1[:])
                    for c in range(LQ):
                        cs = slice(c * 512, (c + 1) * 512)
                        bc0 = nsc.tile([P, 512], f32, tag="bc0")
                        bc1 = nsc.tile([P, 512], f32, tag="bc1")
                        nc.gpsimd.partition_broadcast(bc0[:], r0[:, cs])
                        nc.gpsimd.partition_broadcast(bc1[:], r1[:, cs])
                        seg = qkt[t][:, cs]
                        nc.vector.tensor_mul(seg[0:64, :], seg[0:64, :],
                                             bc0[0:64, :])
                        nc.vector.tensor_mul(seg[64:128, :], seg[64:128, :],
                                             bc1[64:128, :])

            # ---------- phase 2c: v projection (natural, +bias, +ones) ----
            va_pool = octx.enter_context(
                tc.tile_pool(name="vap", bufs=1, side="left"))
            va = []
            with tc.tile_pool(name="wvp", bufs=1, side="right") as wvp, \
                 tc.tile_pool(name="vps", bufs=3, space="PSUM",
                              side="right") as vps:
                wvt = []
                for kk in range(KC):
                    w = wvp.tile([P, GD], f32r, tag=f"wv{kk}", name=f"wv{kk}")
                    nc.sync.dma_start(w[:],
                                      wv[kk * P:(kk + 1) * P, :].bitcast(f32r))
                    wvt.append(w)
                for lk in range(LK):
                    ps = vps.tile([P, GD], f32, tag="vps")
                    for kk in range(KC):
                        nc.tensor.matmul(
                            ps[:], lhsT=xh[kk][:, lk * P:(lk + 1) * P],
                            rhs=wvt[kk][:], start=(kk == 0),
                            stop=(kk == KC - 1))
                    vt = va_pool.tile([P, VA_W], f32r, tag=f"va{lk}",
                                      name=f"va{lk}")
                    grouped = vt[:, 0:HPG * (D + 1)].rearrange(
                        "p (h x) -> p h x", x=D + 1)
                    nc.vector.tensor_add(
                        grouped[:, :, 0:D],
                        ps[:].rearrange("p (h x) -> p h x", x=D),
                        vb_t[:].rearrange("p (h x) -> p h x", x=D))
                    nc.vector.tensor_copy(
                        grouped[:, :, D:D + 1],
                        ones8_t[:].rearrange("p (h x) -> p h x", x=1))
                    va.append(vt)
            if DEBUG:
                for t in range(8):
                    nc.sync.dma_start(
                        dbg["dqkn"][t * P:(t + 1) * P, :],
                        qkt[t][:].bitcast(f32))
                for lk in range(LK):
                    nc.sync.dma_start(
                        dbg["dva"][lk * P:(lk + 1) * P, :],
                        va[lk][:].bitcast(f32))
        # xh freed here

        # ---------- phase 3: rope on q,k ----------
        with tc.tile_pool(name="ropes", bufs=1, side="right") as rp, \
             tc.tile_pool(name="rsc", bufs=2, side="right") as rsc, \
             tc.tile_pool(name="rps", bufs=2, space="PSUM",
                          side="right") as rps:
            rt = []
            for i in range(4):
                r = rp.tile([P, L], f32r, tag=f"rope{i}", name=f"rope{i}")
                nc.sync.dma_start(r[:], ropes[i, :, :].bitcast(f32r))
                rt.append(r)
            for t in range(8):
                cosT = rt[0] if t < 4 else rt[2]
                sinT = rt[1] if t < 4 else rt[3]
                for hf in range(2):
                    sl = slice(hf * 1024, (hf + 1) * 1024)
                    pr = rps.tile([P, 1024], f32, tag="protps")
                    for q2 in range(2):
                        s2 = slice(hf * 1024 + q2 * 512,
                                   hf * 1024 + (q2 + 1) * 512)
                        nc.tensor.matmul(pr[:, q2 * 512:(q2 + 1) * 512],
                                         lhsT=prot_t[:], rhs=qkt[t][:, s2],
                                         start=True, stop=True)
                    t1 = rsc.tile([P, 1024], f32r, tag="t1")
                    nc.vector.tensor_mul(t1[:], qkt[t][:, sl], cosT[:, sl])
                    t2 = rsc.tile([P, 1024], f32r, tag="t2")
                    nc.vector.tensor_mul(t2[:], pr[:], sinT[:, sl])
                    nc.vector.tensor_add(qkt[t][:, sl], t1[:], t2[:])

        if DEBUG:
            for t in range(8):
                nc.sync.dma_start(dbg["dqkr"][t * P:(t + 1) * P, :],
                                  qkt[t][:].bitcast(f32))

        # ---------- phase 4: attention ----------
        on_pool = octx.enter_context(
            tc.tile_pool(name="onorm", bufs=1, side="left"))
        onT = [on_pool.tile([P, L], f32r, tag=f"on{p}", name=f"on{p}")
               for p in range(4)]
        with tc.tile_pool(name="epool", bufs=4, side="right") as epool, \
             tc.tile_pool(name="zpool", bufs=4, side="right") as zpool, \
             tc.tile_pool(name="sps4", bufs=2, space="PSUM",
                          side="right") as sps4, \
             tc.tile_pool(name="ops4", bufs=2, space="PSUM",
                          side="right") as ops4:
            for p4 in range(4):
                qt = qkt[p4]
                kt = qkt[4 + p4]
                h1, h2 = 2 * p4, 2 * p4 + 1
                for lq in range(LQ):
                    lqs = slice(lq * 512, (lq + 1) * 512)
                    o1 = ops4.tile([D + 1, 512], f32, tag="o1")
                    o2 = ops4.tile([D + 1, 512], f32, tag="o2")

                    def emit_pv(et, lk):
                        nc.tensor.matmul(
                            o1[:], lhsT=va[lk][:, h1 * 65:h1 * 65 + 65],
                            rhs=et[:, 0:512],
                            start=(lk == 0), stop=(lk == LK - 1))
                        nc.tensor.matmul(
                            o2[:], lhsT=va[lk][:, h2 * 65:h2 * 65 + 65],
                            rhs=et[:, 512:1024],
                            start=(lk == 0), stop=(lk == LK - 1))

                    pend = None
                    for lk in range(LK):
                        lks = slice(lk * P, (lk + 1) * P)
                        sp = sps4.tile([P, 1024], f32, tag="sp")
                        nc.tensor.matmul(sp[:, 0:512],
                                         lhsT=kt[0:64, lks],
                                         rhs=qt[0:64, lqs],
                                         tile_position=(0, 0),
                                         start=True, stop=True)
                        nc.tensor.matmul(sp[:, 512:1024],
                                         lhsT=kt[64:128, lks],
                                         rhs=qt[64:128, lqs],
                                         tile_position=(64, 0),
                                         start=True, stop=True)
                        et = epool.tile([P, 1024], f32r, tag="et")
                        nc.scalar.activation(et[:], sp[:], AF.Exp,
                                             scale=float(1.0 / np.sqrt(D)))
                        if pend is not None:
                            emit_pv(*pend)
                        pend = (et, lk)
                    emit_pv(*pend)
                    for hh, ops in ((0, o1), (1, o2)):
                        zrow = zpool.tile([1, 512], f32, tag="zrow")
                        nc.vector.tensor_copy(zrow[:], ops[D:D + 1, :])
                        rz = zpool.tile([1, 512], f32, tag="rz")
                        nc.vector.reciprocal_approx_fast(
                            out=rz[:], in_=zrow[:])
                        bz = zpool.tile([64, 512], f32, tag="bz")
                        nc.gpsimd.partition_broadcast(bz[:], rz[:])
                        nc.vector.tensor_mul(
                            onT[p4][hh * 64:(hh + 1) * 64, lqs],
                            ops[0:D, :], bz[:])

        if DEBUG:
            for p in range(4):
                nc.sync.dma_start(dbg["don"][p * P:(p + 1) * P, :],
                                  onT[p][:].bitcast(f32))

        # ---------- phase 5: output projection (natural layout out) -----
        with tc.tile_pool(name="wop", bufs=1, side="right") as wop, \
             tc.tile_pool(name="oevict", bufs=3, side="right") as oevict, \
             tc.tile_pool(name="outps", bufs=2, space="PSUM",
                          side="right") as outps:
            wo = {}
            for kk in range(4):
                for hc in range(2):
                    w = wop.tile([P, 512], f32r, tag=f"wo{kk}_{hc}",
                                 name=f"wo{kk}_{hc}")
                    nc.sync.dma_start(
                        w[:], wout[kk * P:(kk + 1) * P,
                                   hc * 512:(hc + 1) * 512].bitcast(f32r))
                    wo[(kk, hc)] = w
            for lq16 in range(LK):
                for hc in range(2):
                    ps = outps.tile([P, 512], f32, tag="ops")
                    for kk in range(4):
                        nc.tensor.matmul(
                            ps[:], lhsT=onT[kk][:, lq16 * P:(lq16 + 1) * P],
                            rhs=wo[(kk, hc)][:],
                            start=(kk == 0), stop=(kk == 3))
                    oe = oevict.tile([P, 512], f32, tag="oe")
                    nc.vector.tensor_copy(oe[:], ps[:])
                    nc.sync.dma_start(
                        out[lq16 * P:(lq16 + 1) * P,
                            hc * 512:(hc + 1) * 512], oe[:])

    nc.compile()
    return nc


def _host_prep(x, condition, rope, W_ada, b_ada, W_qkv, W_out, q_scale, k_scale):
    """Build the 8 per-core input maps (layout + AdaNorm weight folding)."""
    x = np.asarray(x, np.float32)
    cond = np.asarray(condition, np.float64)[:, 0, :]          # [B, C]
    ada = cond @ np.asarray(W_ada, np.float64) + np.asarray(b_ada, np.float64)
    shift = ada[:, :H]                                          # [B, H]
    scale1 = ada[:, H:] + 1.0                                   # [B, H]

    Wq = np.asarray(W_qkv, np.float64)[:, 0:H]
    Wk = np.asarray(W_qkv, np.float64)[:, H:2 * H]
    Wv = np.asarray(W_qkv, np.float64)[:, 2 * H:3 * H]
    Wo = np.asarray(W_out, np.float32)

    cos = np.asarray(rope, np.float64)[0, 0, :, 0, :]           # [L, D]
    sin = np.asarray(rope, np.float64)[1, 0, :, 0, :]
    qs = np.asarray(q_scale, np.float64)
    ks = np.asarray(k_scale, np.float64)
    qs_sw = qs.reshape(-1, 2)[:, ::-1].ravel()
    ks_sw = ks.reshape(-1, 2)[:, ::-1].ravel()

    def rope_tiles(s, s_sw):
        cT = (cos * s[None, :]).T                                # [D, L]
        sT = (sin * s_sw[None, :]).T
        c2 = np.concatenate([cT, cT], 0).astype(np.float32)      # [128, L]
        s2 = np.concatenate([sT, sT], 0).astype(np.float32)
        return c2, s2

    cq2, sq2 = rope_tiles(qs, qs_sw)
    ck2, sk2 = rope_tiles(ks, ks_sw)
    ropes_q = np.stack([cq2, sq2, ck2, sk2]).astype(np.float32)  # [4,128,L]

    prot = np.zeros((P, P), np.float32)
    for i in range(P // 2):
        prot[2 * i + 1, 2 * i] = -1.0
        prot[2 * i, 2 * i + 1] = 1.0
    bo2 = np.zeros((P, 2), np.float32)
    bo2[0:64, 0] = 1.0
    bo2[64:128, 1] = 1.0
    ones8 = np.ones((P, 8), np.float32)

    in_maps = []
    for core in range(NCORES):
        b, g = divmod(core, HG)
        gsl = slice(g * GD, (g + 1) * GD)
        sc_b = scale1[b][:, None]                               # [H, 1]
        wq_eff = (sc_b * Wq[:, gsl])
        wk_eff = (sc_b * Wk[:, gsl])
        wv_eff = (sc_b * Wv[:, gsl])
        bq = shift[b] @ Wq[:, gsl]
        bk = shift[b] @ Wk[:, gsl]
        bv = shift[b] @ Wv[:, gsl]
        bqk = np.concatenate([bq, bk]).reshape(8, P).T.astype(np.float32)
        in_maps.append({
            "xT": np.ascontiguousarray(x[b].T),
            "wqk": np.ascontiguousarray(
                np.concatenate([wq_eff, wk_eff], 1).astype(np.float32)),
            "wv": np.ascontiguousarray(wv_eff.astype(np.float32)),
            "wout": np.ascontiguousarray(Wo[gsl, :]),
            "bqk": np.ascontiguousarray(bqk),
            "vb": np.broadcast_to(bv.astype(np.float32), (P, GD)).copy(),
            "ropes": ropes_q,
            "prot": prot,
            "bo2": bo2,
            "ones8": ones8,
        })
    return in_maps


def kernel(x, condition, rope, W_ada, b_ada, W_qkv, W_out, q_scale, k_scale,
           _trace=False, _tmpdir=None):
    from concourse import bass_utils

    if _compiled[0] is None:
        _compiled[0] = _build()
    nc = _compiled[0]

    in_maps = _host_prep(x, condition, rope, W_ada, b_ada, W_qkv, W_out,
                         q_scale, k_scale)
    kw = {}
    if _trace:
        kw = {"trace": True, "tmpdir": _tmpdir}
    res = bass_utils.run_bass_kernel_spmd(
        nc, in_maps, core_ids=list(range(NCORES)), **kw)

    full = np.empty((B, L, H), np.float32)
    for b in range(B):
        full[b] = res.results[2 * b]["out"] + res.results[2 * b + 1]["out"]
    if _trace:
        return full, res
    return full

